# revision 15
# baseline (speedup 1.0000x reference)
"""Trainium2 Bass kernel for nn_ContextualAttention (sparse_attention).

Contract: kernel(**inputs) takes FULL numpy inputs and returns the FULL
[2, 256, 48, 48] float32 output. Internally shards across 8 NeuronCores as
(batch b in {0,1}) x (side l/r) x (position-half in {0,1}).

v2 design (vs v1): scores in [l, p] layout (128-part l-tiles, no PE
transposes), exp without max-subtraction (logits <= ~50 for this input
distribution), recon on UNNORMALIZED exp with the softmax 1/denominator
folded into a per-tile scale after recon, on-device transpose-conv
overlap-add into a [C, 26, 50] slab, single-variant bf16 mh/fp inputs with
on-chip wrap-variant construction, all matmuls bf16.

Per-core device work for unit (b, side), half h (288 positions):
  scores[l, p] = sum_{ki,kj,ch} fp[ch, l+off] * mh[ch, p+off]   (18 matmuls)
  Eb[l, p] = exp(scores * (10*invd[l]))          (ACT per-partition scale)
  den[p] = sum_l Eb  (PE ones-matmul);  r = 1/den (DVE)
  po[cf, p] = sum_l rawT[l, cf] * Eb[l, p]       (cf = ch*2048+ij*128+c)
  slab[c, 2y+i, 2x+j] += po * r[p]               (DVE mul + strided add)
Host: downsample, pad/flatten images, rawT reorder+0.25 scale, feature-norm
inv, slab overlap-add across halves, cosine blend.
"""

import sys

for _p in ("/opt/trn_rl_repo", "/root/.axon_site/_ro/trn_rl_repo"):
    if _p not in sys.path:
        sys.path.append(_p)

import numpy as np
import ml_dtypes

BF16 = ml_dtypes.bfloat16

B, C, H, W = 2, 256, 48, 48
HD = WD = 24          # downsampled spatial
L = HD * WD           # 576 filter positions
PH = L // 2           # 288 positions per core (half)
CF = C * 16           # 4096 reconstruction features (ch, ij, c)
EPS_SUM = 2304 * 1e-4  # sum_k (f^2 + eps) = sumsq + K*eps
SCALE = 10.0
MHW = 14 * 24 + 2     # mh row length incl 1-elem zero guards
FPW = 26 * 24 + 2     # fp row length incl guards
SLABW = 26 * 50       # per-half output slab: rows 2y+i in 0..25, cols 2x+j

# l-tiles for the 576-long filter axis: 4x128 + 64
LT = [(0, 128), (128, 128), (256, 128), (384, 128), (512, 64)]

# knob: broadcast r across partitions via stride-0 AP (True) or PE matmul
RBC_PB = False

_CACHED = {}


def _build_nc(reps=1):
    from concourse import bacc, mybir
    from concourse.dt import dt
    from concourse.tile import TileContext

    f32 = dt.float32
    f32r = dt.float32r
    bf16 = dt.bfloat16

    nc = bacc.Bacc("TRN2", target_bir_lowering=False, debug=False,
                   num_devices=8)
    mh_d = nc.declare_dram_parameter("mh1", [C, MHW], bf16, isOutput=False)
    fp_d = nc.declare_dram_parameter("fp1", [C, FPW], bf16, isOutput=False)
    rawT_d = nc.declare_dram_parameter("rawT", [L, CF], bf16, isOutput=False)
    iv_d = nc.declare_dram_parameter("invd10", [L, 1], f32, isOutput=False)
    out_d = nc.declare_dram_parameter("out", [C, SLABW], f32, isOutput=True)

    AF = mybir.ActivationFunctionType

    with TileContext(nc) as tc:
        with (
            tc.tile_pool(name="persist", bufs=1) as pp,
            tc.tile_pool(name="inbuf", bufs=2) as ib,
            tc.tile_pool(name="tmp", bufs=4) as sp,
            tc.tile_pool(name="ps_score", bufs=2, space="PSUM") as ps_s,
            tc.tile_pool(name="ps_den", bufs=1, space="PSUM") as ps_d,
            tc.tile_pool(name="ps_out", bufs=5, space="PSUM") as ps_o,
        ):
          for _rep in range(reps):
              # ---- persistent SBUF tensors + input DMAs ----
              # variant v: 0 -> kj=0 (col 23 zeroed), 1 -> middle, 2 -> kj=2
              mh = [[ib.tile([128, MHW], bf16, tag=f"mh{v}{i}", name=f"mh{v}{i}")
                     for i in range(2)] for v in range(3)]
              fp = [[ib.tile([128, FPW], bf16, tag=f"fp{v}{i}", name=f"fp{v}{i}")
                     for i in range(2)] for v in range(3)]
              rawT = [ib.tile([128, CF], bf16, tag=f"rawT{i}", name=f"rawT{i}")
                      for i in range(5)]
              Eb = [pp.tile([128, PH], bf16, tag=f"Eb{i}", name=f"Eb{i}")
                    for i in range(5)]
              iv = [pp.tile([128, 1], f32, tag=f"iv{i}", name=f"iv{i}")
                    for i in range(5)]
              slab = [pp.tile([128, SLABW], f32, tag=f"slab{i}", name=f"slab{i}")
                      for i in range(2)]
              onesc = pp.tile([128, 1], bf16, tag="onesc", name="onesc")
              rrec = pp.tile([1, PH], f32, tag="rrec", name="rrec")
              rbc = pp.tile([128, PH], f32, tag="rbc", name="rbc")

              # 3 wrap-variants of each image: 3 DMA reads of the same DRAM
              # region (DMA has headroom; engine copies would stall PE at rep
              # boundaries), then zero the contaminated column per variant.
              for ch in range(2):
                  for v in range(3):
                      nc.sync.dma_start(mh[v][ch][:, :],
                                        mh_d[ch * 128:(ch + 1) * 128, :])
                      nc.sync.dma_start(fp[v][ch][:, :],
                                        fp_d[ch * 128:(ch + 1) * 128, :])
              for lt, (l0, lsz) in enumerate(LT):
                  nc.sync.dma_start(iv[lt][0:lsz, :], iv_d[l0:l0 + lsz, :])
              for lt, (l0, lsz) in enumerate(LT):
                  nc.sync.dma_start(rawT[lt][0:lsz, :], rawT_d[l0:l0 + lsz, :])

              nc.vector.memset(onesc[:, :], 1.0)
              for ch in range(2):
                  nc.gpsimd.memset(slab[ch][:, :], 0.0)

              for ch in range(2):
                  # kj=0 variant: zero col x=23; kj=2 variant: zero col x=0
                  # (guard offset 1: row r col x lives at 1 + r*24 + x)
                  for v, x in ((0, 23), (2, 0)):
                      nc.vector.memset(
                          mh[v][ch][:, 1 + x: 1 + x + 13 * 24 + 1: 24], 0.0)
                      nc.vector.memset(
                          fp[v][ch][:, 1 + x: 1 + x + 25 * 24 + 1: 24], 0.0)

              # ---- scores + exp, one l-tile at a time ----
              for lt, (l0, lsz) in enumerate(LT):
                  ps = ps_s.tile([128, PH], f32, tag="ps", name="ps")
                  k = 0
                  for kj in (1, 0, 2):
                      for ki in range(3):
                          for ch in range(2):
                              off = 24 * ki + kj
                              nc.tensor.matmul(
                                  ps[0:lsz, :],
                                  fp[kj][ch][:, l0 + off: l0 + off + lsz],
                                  mh[kj][ch][:, off: off + PH],
                                  start=(k == 0), stop=(k == 17))
                              k += 1
                  nc.scalar.activation(Eb[lt][0:lsz, :], ps[0:lsz, :], AF.Exp,
                                       scale=iv[lt][0:lsz, :])

              # ---- softmax denominator: den[p] = sum_l Eb; r = 1/den ----
              den = ps_d.tile([1, PH], f32, tag="den", name="den")
              for lt, (l0, lsz) in enumerate(LT):
                  nc.tensor.matmul(den[:, :], onesc[0:lsz, :], Eb[lt][0:lsz, :],
                                   start=(lt == 0), stop=(lt == 4))
              nc.vector.reciprocal(rrec[:, :], den[:, :])
              nc.gpsimd.partition_broadcast(rbc[:, :], rrec[:, :])

              # ---- reconstruction + on-chip overlap-add into slab ----
              # po is scaled by r and overlap-added AFTER recon (rbc is ready
              # well before the first po lands -> no PE stall); mul+add pairs
              # alternate DVE/Pool by cf parity. cf block order: ch-major so
              # slab[0] finishes mid-recon and its DMA overlaps the rest.
              for ch in range(2):
                  for ij in range(16):
                      i, j = ij >> 2, ij & 3
                      cf0 = ch * 2048 + ij * 128
                      po = ps_o.tile([128, PH], f32, tag="po", name="po")
                      for lt, (l0, lsz) in enumerate(LT):
                          nc.tensor.matmul(
                              po[:, :],
                              rawT[lt][0:lsz, cf0:cf0 + 128],
                              Eb[lt][0:lsz, :],
                              start=(lt == 0), stop=(lt == 4))
                      # Pool can't read PSUM: mul (PSUM src) on DVE, slab
                      # add (SBUF-only) on Pool.
                      tmp = sp.tile([128, PH], f32, tag="tmp", name="tmp")
                      nc.vector.tensor_mul(tmp[:, :], po[:, :], rbc[:, :])
                      sv = slab[ch].rearrange(
                          "p (r c) -> p r c", r=26, c=50)[:, i:i + 23:2,
                                                          j:j + 47:2]
                      nc.gpsimd.tensor_add(
                          sv, sv,
                          tmp.rearrange("p (y x) -> p y x", y=12, x=24))
                  nc.sync.dma_start(out_d[ch * 128:(ch + 1) * 128, :],
                                    slab[ch][:, :])

    nc.compile()
    return nc


def _prep_inputs(inputs):
    """Build the 8 per-core input maps from the full problem inputs."""
    left = np.asarray(inputs["left"], dtype=np.float32)
    right = np.asarray(inputs["right"], dtype=np.float32)
    mid = np.asarray(inputs["mid"], dtype=np.float32)
    sl = np.asarray(inputs["shortcut_l"], dtype=np.float32)
    sr = np.asarray(inputs["shortcut_r"], dtype=np.float32)

    m_ds = mid[:, :, ::2, ::2]
    f_ds = [left[:, :, ::2, ::2], right[:, :, ::2, ::2]]

    # mh: rows y in [-1, 12] (h=0) / [11, 24] (h=1), zero out-of-range,
    # flattened to 14*24 with 1-elem guards; single middle variant.
    mh1 = np.zeros((B, 2, C, MHW), np.float32)
    for b in range(B):
        for h in range(2):
            m14 = np.zeros((C, 14, 24), np.float32)
            if h == 0:
                m14[:, 1:14] = m_ds[b, :, 0:13]
            else:
                m14[:, 0:13] = m_ds[b, :, 11:24]
            mh1[b, h, :, 1:1 + 14 * 24] = m14.reshape(C, 14 * 24)
    # fp: rows y in [-1, 24]
    fp1 = np.zeros((B, 2, C, FPW), np.float32)
    invd10 = np.zeros((B, 2, L, 1), np.float32)
    for b in range(B):
        for side in range(2):
            f26 = np.zeros((C, 26, 24), np.float32)
            f26[:, 1:25] = f_ds[side][b]
            fp1[b, side, :, 1:1 + 26 * 24] = f26.reshape(C, 26 * 24)
            # host inv_denom: 3x3 window sums of per-pixel channel sumsq
            s = np.zeros((26, 26), np.float32)
            s[1:25, 1:25] = (f_ds[side][b] ** 2).sum(axis=0)
            d2 = np.zeros((24, 24), np.float32)
            for ki in range(3):
                for kj in range(3):
                    d2 += s[ki:ki + 24, kj:kj + 24]
            invd10[b, side] = (SCALE / np.sqrt(d2 + EPS_SUM)).reshape(L, 1)

    def raw_t(s):  # [C,48,48] -> [576, 4096] (l=(y,x), cf=(ch,ij,c)) * 0.25
        p = np.zeros((C, 50, 50), np.float32)
        p[:, 1:49, 1:49] = s
        st = p.strides
        v = np.lib.stride_tricks.as_strided(
            p, shape=(24, 24, C, 4, 4),
            strides=(2 * st[1], 2 * st[2], st[0], st[1], st[2]))
        # (y, x, C, i, j) -> (y, x, ch, i, j, c)
        v6 = v.reshape(24, 24, 2, 128, 4, 4).transpose(0, 1, 2, 4, 5, 3)
        return (np.ascontiguousarray(v6).reshape(L, CF) * 0.25)

    raws = [[raw_t(sl[b]), raw_t(sr[b])] for b in range(B)]

    in_maps = []
    for core in range(8):
        b, side, h = core >> 2, (core >> 1) & 1, core & 1
        in_maps.append({
            "mh1": mh1[b, h].astype(BF16),
            "fp1": fp1[b, side].astype(BF16),
            "rawT": raws[b][side].astype(BF16),
            "invd10": invd10[b, side],
        })
    return in_maps


def _postprocess(results):
    """results: list of 8 dicts with 'out' slab [256, 26*50] -> full output."""
    y = np.zeros((B, 2, C, 48, 48), np.float32)
    for b in range(B):
        for side in range(2):
            acc = np.zeros((C, 50, 50), np.float32)
            s0 = np.asarray(results[(b << 2) | (side << 1) | 0]["out"])
            s1 = np.asarray(results[(b << 2) | (side << 1) | 1]["out"])
            acc[:, 0:26] += s0.reshape(C, 26, 50)
            acc[:, 24:50] += s1.reshape(C, 26, 50)
            y[b, side] = acc[:, 1:49, 1:49]
    j = np.arange(W, dtype=np.float32)
    w = (0.5 * (np.cos(np.pi * j / (W - 1)) + 1.0)).reshape(1, 1, 1, W)
    return w * y[:, 0] + w[..., ::-1] * y[:, 1]


def _run(inputs, trace=False):
    from concourse.bass_utils import run_bass_kernel_spmd

    if "nc" not in _CACHED:
        _CACHED["nc"] = _build_nc()
    in_maps = _prep_inputs(inputs)
    res = run_bass_kernel_spmd(_CACHED["nc"], in_maps, list(range(8)),
                               trace=trace)
    return _postprocess(res.results), res


def kernel(**inputs):
    out, _ = _run(inputs)
    return out


# revision 21
# speedup vs baseline: 11.5464x; 11.5464x over previous
"""Trainium2 Bass kernel for nn_ContextualAttention (sparse_attention).

Contract: kernel(**inputs) takes FULL numpy inputs and returns the FULL
[2, 256, 48, 48] float32 output. Internally shards across 8 NeuronCores as
(batch b in {0,1}) x (side l/r) x (position-half in {0,1}).

v2 design (vs v1): scores in [l, p] layout (128-part l-tiles, no PE
transposes), exp without max-subtraction (logits <= ~50 for this input
distribution), recon on UNNORMALIZED exp with the softmax 1/denominator
folded into a per-tile scale after recon, on-device transpose-conv
overlap-add into a [C, 26, 50] slab, single-variant bf16 mh/fp inputs with
on-chip wrap-variant construction, all matmuls bf16.

Per-core device work for unit (b, side), half h (288 positions):
  scores[l, p] = sum_{ki,kj,ch} fp[ch, l+off] * mh[ch, p+off]   (18 matmuls)
  Eb[l, p] = exp(scores * (10*invd[l]))          (ACT per-partition scale)
  den[p] = sum_l Eb  (PE ones-matmul);  r = 1/den (DVE)
  po[cf, p] = sum_l rawT[l, cf] * Eb[l, p]       (cf = ch*2048+ij*128+c)
  slab[c, 2y+i, 2x+j] += po * r[p]               (DVE mul + strided add)
Host: downsample, pad/flatten images, rawT reorder+0.25 scale, feature-norm
inv, slab overlap-add across halves, cosine blend.
"""

import sys

for _p in ("/opt/trn_rl_repo", "/root/.axon_site/_ro/trn_rl_repo"):
    if _p not in sys.path:
        sys.path.append(_p)

import numpy as np
import ml_dtypes

BF16 = ml_dtypes.bfloat16

B, C, H, W = 2, 256, 48, 48
HD = WD = 24          # downsampled spatial
L = HD * WD           # 576 filter positions
PH = L // 2           # 288 positions per core (half)
CF = C * 16           # 4096 reconstruction features (ch, ij, c)
EPS_SUM = 2304 * 1e-4  # sum_k (f^2 + eps) = sumsq + K*eps
SCALE = 10.0
MHW = 14 * 24 + 2     # mh row length incl 1-elem zero guards
FPW = 26 * 24 + 2     # fp row length incl guards
SLABW = 26 * 50       # per-half output slab: rows 2y+i in 0..25, cols 2x+j

# l-tiles for the 576-long filter axis: 4x128 + 64
LT = [(0, 128), (128, 128), (256, 128), (384, 128), (512, 64)]

# knob: broadcast r across partitions via stride-0 AP (True) or PE matmul
RBC_PB = False

_CACHED = {}


def _build_nc(reps=1):
    from concourse import bacc, mybir
    from concourse.dt import dt
    from concourse.tile import TileContext

    f32 = dt.float32
    f32r = dt.float32r
    bf16 = dt.bfloat16

    nc = bacc.Bacc("TRN2", target_bir_lowering=False, debug=False,
                   num_devices=8)
    mh_d = nc.declare_dram_parameter("mh1", [C, MHW], bf16, isOutput=False)
    fp_d = nc.declare_dram_parameter("fp1", [C, FPW], bf16, isOutput=False)
    rawT_d = nc.declare_dram_parameter("rawT", [L, CF], bf16, isOutput=False)
    iv_d = nc.declare_dram_parameter("invd10", [L, 1], f32, isOutput=False)
    out_d = nc.declare_dram_parameter("out", [C, SLABW], bf16, isOutput=True)

    AF = mybir.ActivationFunctionType

    with TileContext(nc) as tc:
        with (
            tc.tile_pool(name="persist", bufs=1) as pp,
            tc.tile_pool(name="inbuf", bufs=2) as ib,
            tc.tile_pool(name="tmp", bufs=4) as sp,
            tc.tile_pool(name="ps_score", bufs=2, space="PSUM") as ps_s,
            tc.tile_pool(name="ps_den", bufs=1, space="PSUM") as ps_d,
            tc.tile_pool(name="ps_out", bufs=5, space="PSUM") as ps_o,
        ):
          for _rep in range(reps):
              # ---- persistent SBUF tensors + input DMAs ----
              # variant v: 0 -> kj=0 (col 23 zeroed), 1 -> middle, 2 -> kj=2
              mh = [[ib.tile([128, MHW], bf16, tag=f"mh{v}{i}", name=f"mh{v}{i}")
                     for i in range(2)] for v in range(3)]
              fp = [[ib.tile([128, FPW], bf16, tag=f"fp{v}{i}", name=f"fp{v}{i}")
                     for i in range(2)] for v in range(3)]
              rawT = [ib.tile([128, CF], bf16, tag=f"rawT{i}", name=f"rawT{i}")
                      for i in range(5)]
              Eb = [pp.tile([128, PH], bf16, tag=f"Eb{i}", name=f"Eb{i}")
                    for i in range(5)]
              iv = [pp.tile([128, 1], f32, tag=f"iv{i}", name=f"iv{i}")
                    for i in range(5)]
              slab = [pp.tile([128, SLABW], bf16, tag=f"slab{i}", name=f"slab{i}")
                      for i in range(2)]
              onesc = pp.tile([128, 1], bf16, tag="onesc", name="onesc")
              rrec = pp.tile([1, PH], f32, tag="rrec", name="rrec")
              rbc = pp.tile([128, PH], f32, tag="rbc", name="rbc")

              # 3 wrap-variants of each image: 3 DMA reads of the same DRAM
              # region (DMA has headroom; engine copies would stall PE at rep
              # boundaries), then zero the contaminated column per variant.
              for ch in range(2):
                  for v in range(3):
                      nc.sync.dma_start(mh[v][ch][:, :],
                                        mh_d[ch * 128:(ch + 1) * 128, :])
                      nc.sync.dma_start(fp[v][ch][:, :],
                                        fp_d[ch * 128:(ch + 1) * 128, :])
              for lt, (l0, lsz) in enumerate(LT):
                  nc.sync.dma_start(iv[lt][0:lsz, :], iv_d[l0:l0 + lsz, :])
              for lt, (l0, lsz) in enumerate(LT):
                  nc.sync.dma_start(rawT[lt][0:lsz, :], rawT_d[l0:l0 + lsz, :])

              nc.vector.memset(onesc[:, :], 1.0)
              for ch in range(2):
                  nc.scalar.memzero(slab[ch][:, :])

              for ch in range(2):
                  # kj=0 variant: zero col x=23; kj=2 variant: zero col x=0
                  # (guard offset 1: row r col x lives at 1 + r*24 + x)
                  for v, x in ((0, 23), (2, 0)):
                      nc.vector.memset(
                          mh[v][ch][:, 1 + x: 1 + x + 13 * 24 + 1: 24], 0.0)
                      nc.vector.memset(
                          fp[v][ch][:, 1 + x: 1 + x + 25 * 24 + 1: 24], 0.0)

              # ---- scores + exp, one l-tile at a time ----
              for lt, (l0, lsz) in enumerate(LT):
                  ps = ps_s.tile([128, PH], f32, tag="ps", name="ps")
                  k = 0
                  for kj in (1, 0, 2):
                      for ki in range(3):
                          for ch in range(2):
                              off = 24 * ki + kj
                              nc.tensor.matmul(
                                  ps[0:lsz, :],
                                  fp[kj][ch][:, l0 + off: l0 + off + lsz],
                                  mh[kj][ch][:, off: off + PH],
                                  start=(k == 0), stop=(k == 17))
                              k += 1
                  nc.scalar.activation(Eb[lt][0:lsz, :], ps[0:lsz, :], AF.Exp,
                                       scale=iv[lt][0:lsz, :])

              # ---- softmax denominator: den[p] = sum_l Eb; r = 1/den ----
              den = ps_d.tile([1, PH], f32, tag="den", name="den")
              for lt, (l0, lsz) in enumerate(LT):
                  nc.tensor.matmul(den[:, :], onesc[0:lsz, :], Eb[lt][0:lsz, :],
                                   start=(lt == 0), stop=(lt == 4))
              nc.vector.reciprocal(rrec[:, :], den[:, :])
              nc.gpsimd.partition_broadcast(rbc[:, :], rrec[:, :])

              # ---- reconstruction + on-chip overlap-add into slab ----
              # po is scaled by r and overlap-added AFTER recon (rbc is ready
              # well before the first po lands -> no PE stall); mul+add pairs
              # alternate DVE/Pool by cf parity. cf block order: ch-major so
              # slab[0] finishes mid-recon and its DMA overlaps the rest.
              for ch in range(2):
                  for ij in range(16):
                      i, j = ij >> 2, ij & 3
                      cf0 = ch * 2048 + ij * 128
                      po = ps_o.tile([128, PH], f32, tag="po", name="po")
                      for lt, (l0, lsz) in enumerate(LT):
                          nc.tensor.matmul(
                              po[:, :],
                              rawT[lt][0:lsz, cf0:cf0 + 128],
                              Eb[lt][0:lsz, :],
                              start=(lt == 0), stop=(lt == 4))
                      # GPSIMD TensorTensor is ~3us/op on HW (ucode) — keep
                      # all elementwise on DVE; Pool only does the broadcast.
                      tmp = sp.tile([128, PH], bf16, tag="tmp", name="tmp")
                      nc.vector.tensor_mul(tmp[:, :], po[:, :], rbc[:, :])
                      sv = slab[ch].rearrange(
                          "p (r c) -> p r c", r=26, c=50)[:, i:i + 23:2,
                                                          j:j + 47:2]
                      nc.vector.tensor_add(
                          sv, sv,
                          tmp.rearrange("p (y x) -> p y x", y=12, x=24))
                  nc.sync.dma_start(out_d[ch * 128:(ch + 1) * 128, :],
                                    slab[ch][:, :])

    nc.compile()
    return nc


def _prep_inputs(inputs):
    """Build the 8 per-core input maps from the full problem inputs."""
    left = np.asarray(inputs["left"], dtype=np.float32)
    right = np.asarray(inputs["right"], dtype=np.float32)
    mid = np.asarray(inputs["mid"], dtype=np.float32)
    sl = np.asarray(inputs["shortcut_l"], dtype=np.float32)
    sr = np.asarray(inputs["shortcut_r"], dtype=np.float32)

    m_ds = mid[:, :, ::2, ::2]
    f_ds = [left[:, :, ::2, ::2], right[:, :, ::2, ::2]]

    # mh: rows y in [-1, 12] (h=0) / [11, 24] (h=1), zero out-of-range,
    # flattened to 14*24 with 1-elem guards; single middle variant.
    mh1 = np.zeros((B, 2, C, MHW), np.float32)
    for b in range(B):
        for h in range(2):
            m14 = np.zeros((C, 14, 24), np.float32)
            if h == 0:
                m14[:, 1:14] = m_ds[b, :, 0:13]
            else:
                m14[:, 0:13] = m_ds[b, :, 11:24]
            mh1[b, h, :, 1:1 + 14 * 24] = m14.reshape(C, 14 * 24)
    # fp: rows y in [-1, 24]
    fp1 = np.zeros((B, 2, C, FPW), np.float32)
    invd10 = np.zeros((B, 2, L, 1), np.float32)
    for b in range(B):
        for side in range(2):
            f26 = np.zeros((C, 26, 24), np.float32)
            f26[:, 1:25] = f_ds[side][b]
            fp1[b, side, :, 1:1 + 26 * 24] = f26.reshape(C, 26 * 24)
            # host inv_denom: 3x3 window sums of per-pixel channel sumsq
            s = np.zeros((26, 26), np.float32)
            s[1:25, 1:25] = (f_ds[side][b] ** 2).sum(axis=0)
            d2 = np.zeros((24, 24), np.float32)
            for ki in range(3):
                for kj in range(3):
                    d2 += s[ki:ki + 24, kj:kj + 24]
            invd10[b, side] = (SCALE / np.sqrt(d2 + EPS_SUM)).reshape(L, 1)

    def raw_t(s):  # [C,48,48] -> [576, 4096] (l=(y,x), cf=(ch,ij,c)) * 0.25
        p = np.zeros((C, 50, 50), np.float32)
        p[:, 1:49, 1:49] = s
        st = p.strides
        v = np.lib.stride_tricks.as_strided(
            p, shape=(24, 24, C, 4, 4),
            strides=(2 * st[1], 2 * st[2], st[0], st[1], st[2]))
        # (y, x, C, i, j) -> (y, x, ch, i, j, c)
        v6 = v.reshape(24, 24, 2, 128, 4, 4).transpose(0, 1, 2, 4, 5, 3)
        return (np.ascontiguousarray(v6).reshape(L, CF) * 0.25)

    raws = [[raw_t(sl[b]), raw_t(sr[b])] for b in range(B)]

    in_maps = []
    for core in range(8):
        b, side, h = core >> 2, (core >> 1) & 1, core & 1
        in_maps.append({
            "mh1": mh1[b, h].astype(BF16),
            "fp1": fp1[b, side].astype(BF16),
            "rawT": raws[b][side].astype(BF16),
            "invd10": invd10[b, side],
        })
    return in_maps


def _postprocess(results):
    """results: list of 8 dicts with 'out' slab [256, 26*50] -> full output."""
    y = np.zeros((B, 2, C, 48, 48), np.float32)
    for b in range(B):
        for side in range(2):
            acc = np.zeros((C, 50, 50), np.float32)
            s0 = np.asarray(results[(b << 2) | (side << 1) | 0]["out"],
                            dtype=np.float32)
            s1 = np.asarray(results[(b << 2) | (side << 1) | 1]["out"],
                            dtype=np.float32)
            acc[:, 0:26] += s0.reshape(C, 26, 50)
            acc[:, 24:50] += s1.reshape(C, 26, 50)
            y[b, side] = acc[:, 1:49, 1:49]
    j = np.arange(W, dtype=np.float32)
    w = (0.5 * (np.cos(np.pi * j / (W - 1)) + 1.0)).reshape(1, 1, 1, W)
    return w * y[:, 0] + w[..., ::-1] * y[:, 1]


def _run(inputs, trace=False):
    from concourse.bass_utils import run_bass_kernel_spmd

    if "nc" not in _CACHED:
        _CACHED["nc"] = _build_nc()
    in_maps = _prep_inputs(inputs)
    res = run_bass_kernel_spmd(_CACHED["nc"], in_maps, list(range(8)),
                               trace=trace)
    return _postprocess(res.results), res


def kernel(**inputs):
    out, _ = _run(inputs)
    return out
